# revision 1
# baseline (speedup 1.0000x reference)
"""Trainium Bass kernel for nn_Network_44968307589213 (RandLA-Net style
dilated residual block: neighbor gathers + relative-pos encoding + two
attentive pools), SPMD across 8 NeuronCores.

Sharding: batch (2) x point-quarter (4): core c handles batch c//4,
points [ (c%4)*8192, (c%4+1)*8192 ). Two NEFF launches: launch 1 computes
f_pc / rel-pos encoding / attention-1 per quarter; the per-quarter f_agg
outputs are concatenated on host (pure layout) and fed to launch 2, which
runs attention-2 and the final mlp2+shortcut. Neighbor rows are fetched
with dma_gather (transpose mode, 256B rows) from per-core DRAM tables.
"""
import os
import numpy as np
import ml_dtypes

import numpy as np
import concourse.bass as bass
import concourse.mybir as mybir
import concourse.tile as tile
import concourse.bass_isa as bass_isa
from concourse import bacc

FP = mybir.dt.float32
BF = mybir.dt.bfloat16
I16 = mybir.dt.int16
AX = mybir.AxisListType
ALU = mybir.AluOpType
ACT_T = mybir.ActivationFunctionType


def build_launch1(N=32768, NL=8192, K=16, CH=4096, NQ=4, level=6):
    """CH = edges per gather chunk (multiple of 128). NL*K/CH chunks."""
    NCH = NL * K // CH          # chunks
    PTS = CH // K               # points per chunk
    NB = N // 128               # 128-row blocks in table

    nc = bacc.Bacc(name="randla1", num_swdge_queues=NQ)
    feat33 = nc.dram_tensor("feat33", [33, N], FP, kind="ExternalInput")
    xyz_rows = nc.dram_tensor("xyz_rows", [N, 3], FP, kind="ExternalInput")
    xyzT1 = nc.dram_tensor("xyzT1", [4, N], BF, kind="ExternalInput")
    idxw = nc.dram_tensor("idxw", [128, NL], I16, kind="ExternalInput")
    w1aug = nc.dram_tensor("w1aug", [33, 32], FP, kind="ExternalInput")
    vT = nc.dram_tensor("vT", [3, 32], BF, kind="ExternalInput")
    uaug = nc.dram_tensor("uaug", [4, 32], BF, kind="ExternalInput")
    wd = nc.dram_tensor("wd", [32, 1], FP, kind="ExternalInput")
    fc1T = nc.dram_tensor("fc1T", [64, 64], BF, kind="ExternalInput")
    wa1T = nc.dram_tensor("wa1T", [64, 32], BF, kind="ExternalInput")
    ba1 = nc.dram_tensor("ba1", [32, 1], FP, kind="ExternalInput")

    f_agg_o = nc.dram_tensor("f_agg", [32, NL], FP, kind="ExternalOutput")
    fxyz_o = nc.dram_tensor("fxyz", [32, NL * K], BF, kind="ExternalOutput")

    # scratch DRAM
    t1 = nc.dram_tensor("t1d", [N, 128], BF)                  # gather table
    xtsq_d = nc.dram_tensor("xtsqd", [N], FP)                 # |xyz|^2 row-major
    bounce = nc.dram_tensor("bounced", [NCH, CH], FP)         # cross bounce
    bounce_bf = nc.dram_tensor("bouncebf", [NCH, 2, CH], BF)  # nsq/dist bounce

    with tile.TileContext(nc) as tc:
        with tc.tile_pool(name="w", bufs=1) as wp:
            w1_sb = wp.tile([33, 32], FP)
            nc.sync.dma_start(w1_sb[:], w1aug[:])
            vT_sb = wp.tile([3, 32], BF)
            nc.sync.dma_start(vT_sb[:], vT[:])
            uaug_sb = wp.tile([36, 32], BF)
            nc.sync.dma_start(uaug_sb[32:36, :], uaug[:])
            wd_sb = wp.tile([32, 1], FP)
            nc.sync.dma_start(wd_sb[:], wd[:])
            fc1_sb = wp.tile([64, 64], BF)
            nc.sync.dma_start(fc1_sb[:], fc1T[:])
            wa1_sb = wp.tile([64, 32], BF)
            nc.sync.dma_start(wa1_sb[:], wa1T[:])
            ba1_sb = wp.tile([32, 1], FP)
            nc.sync.dma_start(ba1_sb[:], ba1[:])
            idx_sb = wp.tile([128, NL], I16)
            nc.sync.dma_start(idx_sb[:], idxw[:])

            # ---------- stage A: build T1 ----------
            # f_pc: per 512-col tile: psum = w1aug^T @ feat33 -> relu bf16
            with tc.tile_pool(name="sa", bufs=3) as sa, \
                 tc.tile_pool(name="sap", bufs=4, space="PSUM") as sap:
                TS = 512
                for i in range(N // TS):
                    ft = sa.tile([33, TS], FP, tag="ft")
                    nc.sync.dma_start(ft[:], feat33[:, i * TS:(i + 1) * TS])
                    ps = sap.tile([32, TS], FP, tag="ps")
                    nc.tensor.matmul(ps[:], w1_sb[:], ft[:], start=True, stop=True)
                    fpc = sa.tile([32, TS], BF, tag="fpc")
                    nc.scalar.activation(fpc[:], ps[:], ACT_T.Relu)
                    # transpose [32, TS] -> [128, TS//128, 32] then write T1[:, 0:32]
                    tr = sa.tile([128, TS // 128, 32], BF, tag="tr")
                    nc.sync.dma_start_transpose(tr[:], fpc[:])
                    nc.sync.dma_start(
                        t1[:, 0:32].rearrange("(b p) c -> p b c", p=128)[:, i * (TS // 128):(i + 1) * (TS // 128), :],
                        tr[:])
                    # v = C^T-matmul: psum2 = vT^T?? v = C@xyz: lhsT=vT_sb [3,32], rhs=xyzT1[0:3] bf16
                    xt = sa.tile([4, TS], BF, tag="xt")
                    nc.sync.dma_start(xt[:], xyzT1[:, i * TS:(i + 1) * TS])
                    ps2 = sap.tile([32, TS], FP, tag="ps2")
                    nc.tensor.matmul(ps2[:], vT_sb[:], xt[0:3, :], start=True, stop=True)
                    vv = sa.tile([32, TS], BF, tag="vv")
                    nc.vector.tensor_copy(vv[:], ps2[:])
                    tr2 = sa.tile([128, TS // 128, 32], BF, tag="tr2")
                    nc.sync.dma_start_transpose(tr2[:], vv[:])
                    nc.sync.dma_start(
                        t1[:, 64:96].rearrange("(b p) c -> p b c", p=128)[:, i * (TS // 128):(i + 1) * (TS // 128), :],
                        tr2[:])
                # zero-fill pad channels 68:128 of t1
                BB = min(32, NB)
                zt = wp.tile([128, BB, 60], BF)
                nc.vector.memset(zt[:], 0.0)
                for i in range(NB // BB):
                    nc.sync.dma_start(
                        t1[:, 36:64].rearrange("(b p) c -> p b c", p=128)[:, i * BB:(i + 1) * BB, :],
                        zt[:, :, 0:28])
                    nc.sync.dma_start(
                        t1[:, 96:128].rearrange("(b p) c -> p b c", p=128)[:, i * BB:(i + 1) * BB, :],
                        zt[:, :, 28:60])
                # geo channels: xyz rows + |xyz|^2, blocks of 128 rows x BB
                for i in range(NB // BB):
                    xr = sa.tile([128, BB, 3], FP, tag="xr")
                    nc.sync.dma_start(
                        xr[:],
                        xyz_rows[:].rearrange("(b p) c -> p b c", p=128)[:, i * BB:(i + 1) * BB, :])
                    sq = sa.tile([128, BB, 3], FP, tag="sq")
                    nc.vector.tensor_tensor(sq[:], xr[:], xr[:], ALU.mult)
                    nsq = sa.tile([128, BB], FP, tag="nsq")
                    nc.vector.tensor_reduce(nsq[:], sq[:], axis=AX.X, op=ALU.add)
                    geo = sa.tile([128, BB, 4], BF, tag="geo")
                    nc.vector.tensor_copy(geo[:, :, 0:3], xr[:])
                    nc.vector.tensor_copy(geo[:, :, 3:4], nsq[:].unsqueeze(2))
                    nc.sync.dma_start(
                        t1[:, 32:36].rearrange("(b p) c -> p b c", p=128)[:, i * BB:(i + 1) * BB, :],
                        geo[:])
                    # |xyz|^2 back to DRAM row-major for spread-bcast reads
                    nc.sync.dma_start(
                        xtsq_d[:].rearrange("(b p) -> p b", p=128)[:, i * BB:(i + 1) * BB],
                        nsq[:])

            # ---------- stage 1 chunks ----------
            NCH_eff = 0 if level == 0 else NCH
            n0 = 0  # local chunk base (points); edges base = t*CH
            with tc.tile_pool(name="c", bufs=2) as cp, \
                 tc.tile_pool(name="cp2", bufs=2) as cp2, \
                 tc.tile_pool(name="pp", bufs=2, space="PSUM") as pp:
                for t in range(NCH_eff):
                    e0 = t * CH
                    p0 = t * PTS
                    G = cp.tile([128, CH], BF, tag="G")
                    nc.gpsimd.dma_gather(
                        G[:].unsqueeze(1), t1[:],
                        idx_sb[:, e0 // 16:(e0 + CH) // 16],
                        num_idxs=CH, num_idxs_reg=CH, elem_size=128,
                        transpose=True, single_packet=False, queue_num=t % NQ)
                    if level == 1:
                        dum = cp.tile([32, PTS], FP, tag="dummy", name="dum")
                        nc.vector.tensor_copy(dum[:], G[0:32, 0:PTS])
                        continue
                    # xt chunk [4, PTS] bf16
                    xtc = cp.tile([36, PTS], BF, tag="xtc")
                    nc.sync.dma_start(xtc[32:36, :], xyzT1[:, p0:p0 + PTS])
                    # cross = sum_d xt_d*nx_d ; mult into [3, CH] then PAR
                    crm = cp.tile([3, CH], BF, tag="crm")
                    nc.vector.tensor_tensor(
                        crm[:].rearrange("p (n k) -> p n k", k=K),
                        xtc[32:35, :].unsqueeze(2).broadcast_to([3, PTS, K]),
                        G[32:35, :].rearrange("p (n k) -> p n k", k=K),
                        ALU.mult)
                    crs = cp.tile([3, CH], FP, tag="crs", bufs=1)
                    nc.gpsimd.partition_all_reduce(crs[:], crm[:], channels=3,
                                                   reduce_op=bass_isa.ReduceOp.add)
                    # spread cross & |nx|2 via DRAM bounce; |xt|2 direct from DRAM
                    nc.sync.dma_start(bounce[t, :].unsqueeze(0), crs[0:1, :])
                    SPW = CH // 128
                    csp = cp.tile([128, SPW], FP, tag="csp")
                    nc.sync.dma_start(csp[:], bounce[t, :].rearrange("(p f) -> p f", p=128))
                    nc.sync.dma_start(bounce_bf[t, 0, :].unsqueeze(0), G[35:36, :])
                    nsp = cp.tile([128, SPW], BF, tag="nsp")
                    nc.sync.dma_start(nsp[:], bounce_bf[t, 0, :].rearrange("(p f) -> p f", p=128))
                    xs2 = cp.tile([128, PTS // 128], FP, tag="xs2")
                    nc.sync.dma_start(
                        xs2[:], xtsq_d[p0:p0 + PTS].rearrange("(p a) -> p a", p=128))
                    xsp = cp.tile([128, SPW], FP, tag="xsp")
                    nc.vector.tensor_copy(
                        xsp[:].rearrange("p (a k) -> p a k", k=K),
                        xs2[:].unsqueeze(2).broadcast_to([128, PTS // 128, K]))
                    if level == 2:
                        continue
                    # d2 = xsp - 2*crs + nsq ; dist = exp(.5 ln(max(d2,eps)))
                    d2 = cp.tile([128, SPW], FP, tag="d2")
                    nc.vector.scalar_tensor_tensor(d2[:], csp[:], -2.0, xsp[:],
                                                   op0=ALU.mult, op1=ALU.add)
                    nc.vector.tensor_tensor(d2[:], d2[:], nsp[:], ALU.add)
                    nc.vector.tensor_scalar(d2[:], d2[:], 1e-12, None, ALU.max)
                    nc.scalar.activation(d2[:], d2[:], ACT_T.Ln)
                    dsp = cp.tile([128, SPW], BF, tag="dsp")
                    nc.scalar.activation(dsp[:], d2[:], ACT_T.Exp, scale=0.5)
                    # unspread via bounce -> [1, CH] -> pbcast [32, CH]
                    nc.sync.dma_start(bounce_bf[t, 1, :].rearrange("(p f) -> p f", p=128), dsp[:])
                    dr = cp.tile([1, CH], BF, tag="dr")
                    nc.sync.dma_start(dr[:], bounce_bf[t, 1, :].unsqueeze(0))
                    db = cp.tile([32, CH], BF, tag="db", bufs=1)
                    nc.gpsimd.partition_broadcast(db[:], dr[:], channels=32)
                    if level == 3:
                        continue
                    # u_pt = uaug^T @ xtc -> [32, PTS] (enc bias folded via ones row)
                    ups = pp.tile([32, PTS], FP, tag="ups", bufs=1)
                    nc.tensor.matmul(ups[:], uaug_sb[32:36, :], xtc[32:36, :], start=True, stop=True)
                    upt = cp.tile([96, PTS], BF, tag="upt")
                    nc.vector.tensor_copy(upt[64:96, :], ups[:])
                    # cat tile: [64, CH] = [f_nb; f_xyz]
                    cat = cp2.tile([64, CH], BF, tag="cat")
                    nc.vector.tensor_copy(cat[0:32, :], G[0:32, :])
                    # pre = v + u_pt(bcast); pre = db*wd + pre; relu
                    pre = cp.tile([32, CH], BF, tag="pre", bufs=1)
                    nc.vector.tensor_tensor(
                        pre[:].rearrange("p (n k) -> p n k", k=K),
                        G[64:96, :].rearrange("p (n k) -> p n k", k=K),
                        upt[64:96, :].unsqueeze(2).broadcast_to([32, PTS, K]),
                        ALU.add)
                    pre2 = cp.tile([32, CH], BF, tag="pre2", bufs=1)
                    nc.vector.scalar_tensor_tensor(pre2[:], db[:], wd_sb[:], pre[:],
                                                   op0=ALU.mult, op1=ALU.add)
                    nc.vector.tensor_scalar(cat[32:64, :], pre2[:], 0.0, None, ALU.max)
                    # spill f_xyz
                    nc.sync.dma_start(fxyz_o[:, e0:e0 + CH], cat[32:64, :])
                    if level == 4:
                        continue
                    # logits: fc1a^T@f_nb + fc1b^T@f_xyz per 1024-col slab; exp
                    e_t = cp2.tile([64, CH], BF, tag="e")
                    SL = 1024
                    for s in range(CH // SL):
                        lp = pp.tile([64, SL], FP, tag="lp")
                        for ss in range(SL // 512):
                            sl = slice(s * SL + ss * 512, s * SL + (ss + 1) * 512)
                            pslab = lp[:, ss * 512:(ss + 1) * 512]
                            nc.tensor.matmul(pslab, fc1_sb[:, :], cat[:, sl],
                                             start=True, stop=True)
                        nc.scalar.activation(e_t[:, s * SL:(s + 1) * SL], lp[:], ACT_T.Exp)
                    if level == 5:
                        continue
                    # weighted sums
                    mt = cp2.tile([64, CH], BF, tag="mt", bufs=1)
                    nc.vector.tensor_tensor(mt[:], cat[:], e_t[:], ALU.mult)
                    agg = cp.tile([64, PTS], FP, tag="agg")
                    nc.vector.tensor_reduce(agg[:], mt[:].rearrange("p (n k) -> p n k", k=K),
                                            axis=AX.X, op=ALU.add)
                    se = cp.tile([64, PTS], FP, tag="se")
                    nc.vector.tensor_reduce(se[:], e_t[:].rearrange("p (n k) -> p n k", k=K),
                                            axis=AX.X, op=ALU.add)
                    rse = cp.tile([64, PTS], FP, tag="rse")
                    nc.vector.reciprocal(rse[:], se[:])
                    aggn = cp.tile([64, PTS], BF, tag="aggn")
                    nc.vector.tensor_tensor(aggn[:], agg[:], rse[:], ALU.mult)
                    # att1 mlp: wa1T^T @ aggn + ba1, relu -> f_agg [32, PTS]
                    fps = pp.tile([32, PTS], FP, tag="fps", bufs=1)
                    nc.tensor.matmul(fps[:], wa1_sb[:], aggn[:], start=True, stop=True)
                    fago = cp.tile([32, PTS], FP, tag="fago")
                    nc.scalar.activation(fago[:], fps[:], ACT_T.Relu, bias=ba1_sb[:])
                    nc.sync.dma_start(f_agg_o[:, p0:p0 + PTS], fago[:])
    nc.finalize()
    return nc


def build_launch2(N=32768, NL=8192, K=16, CH=4096, NQ=4):
    NCH = NL * K // CH
    PTS = CH // K

    nc = bacc.Bacc(name="randla2", num_swdge_queues=NQ)
    faggT = nc.dram_tensor("faggT", [32, N], FP, kind="ExternalInput")
    fxyz_i = nc.dram_tensor("fxyz", [32, NL * K], BF, kind="ExternalInput")
    feat33 = nc.dram_tensor("feat33l", [33, NL], FP, kind="ExternalInput")
    idxw = nc.dram_tensor("idxw", [128, NL], I16, kind="ExternalInput")
    wb2T = nc.dram_tensor("wb2T", [32, 32], BF, kind="ExternalInput")
    bb2 = nc.dram_tensor("bb2", [32, 1], FP, kind="ExternalInput")
    fc2T = nc.dram_tensor("fc2T", [64, 64], BF, kind="ExternalInput")
    wa2T = nc.dram_tensor("wa2T", [64, 64], BF, kind="ExternalInput")
    ba2 = nc.dram_tensor("ba2", [64, 1], FP, kind="ExternalInput")
    wm2T = nc.dram_tensor("wm2T", [64, 128], BF, kind="ExternalInput")
    wscaug = nc.dram_tensor("wscaug", [33, 128], FP, kind="ExternalInput")

    out_o = nc.dram_tensor("out", [128, NL], FP, kind="ExternalOutput")
    t2 = nc.dram_tensor("t2d", [N, 128], BF)

    with tile.TileContext(nc) as tc:
        with tc.tile_pool(name="w", bufs=1) as wp:
            wb2_sb = wp.tile([32, 32], BF)
            nc.sync.dma_start(wb2_sb[:], wb2T[:])
            bb2_sb = wp.tile([32, 1], FP)
            nc.sync.dma_start(bb2_sb[:], bb2[:])
            fc2_sb = wp.tile([64, 64], BF)
            nc.sync.dma_start(fc2_sb[:], fc2T[:])
            wa2_sb = wp.tile([64, 64], BF)
            nc.sync.dma_start(wa2_sb[:], wa2T[:])
            ba2_sb = wp.tile([64, 1], FP)
            nc.sync.dma_start(ba2_sb[:], ba2[:])
            wmsc_sb = wp.tile([97, 128], BF)
            nc.sync.dma_start(wmsc_sb[0:64, :], wm2T[:])
            wsc_sb = wp.tile([33, 128], FP)
            nc.sync.dma_start(wsc_sb[:], wscaug[:])
            nc.vector.tensor_copy(wmsc_sb[64:97, :], wsc_sb[:])
            idx_sb = wp.tile([128, NL], I16)
            nc.sync.dma_start(idx_sb[:], idxw[:])

            # ---------- build T2 from faggT ----------
            with tc.tile_pool(name="sa", bufs=3) as sa:
                TS = 512
                NB2 = N // 128
                BB = min(32, NB2)
                zt = wp.tile([128, BB, 96], BF)
                nc.vector.memset(zt[:], 0.0)
                for i in range(NB2 // BB):
                    nc.sync.dma_start(
                        t2[:, 32:128].rearrange("(b p) c -> p b c", p=128)[:, i * BB:(i + 1) * BB, :],
                        zt[:])
                for i in range(N // TS):
                    fa = sa.tile([32, TS], FP, tag="fa")
                    nc.sync.dma_start(fa[:], faggT[:, i * TS:(i + 1) * TS])
                    fb = sa.tile([32, TS], BF, tag="fb")
                    nc.vector.tensor_copy(fb[:], fa[:])
                    tr = sa.tile([128, TS // 128, 32], BF, tag="tr")
                    nc.sync.dma_start_transpose(tr[:], fb[:])
                    nc.sync.dma_start(
                        t2[:, 0:32].rearrange("(b p) c -> p b c", p=128)[:, i * (TS // 128):(i + 1) * (TS // 128), :],
                        tr[:])

            # ---------- stage 2 chunks ----------
            with tc.tile_pool(name="c", bufs=2) as cp, \
                 tc.tile_pool(name="cp2", bufs=2) as cp2, \
                 tc.tile_pool(name="pp", bufs=2, space="PSUM") as pp:
                for t in range(NCH):
                    e0 = t * CH
                    p0 = t * PTS
                    G = cp.tile([128, CH], BF, tag="G")
                    nc.gpsimd.dma_gather(
                        G[:].unsqueeze(1), t2[:],
                        idx_sb[:, e0 // 16:(e0 + CH) // 16],
                        num_idxs=CH, num_idxs_reg=CH, elem_size=128,
                        transpose=True, single_packet=False, queue_num=t % NQ)
                    fx = cp.tile([32, CH], BF, tag="fx")
                    nc.sync.dma_start(fx[:], fxyz_i[:, e0:e0 + CH])
                    cat = cp2.tile([64, CH], BF, tag="cat")
                    nc.vector.tensor_copy(cat[0:32, :], G[0:32, :])
                    # f_xyz2 = relu(wb2@fx + bb2) per 512 slab
                    for s in range(CH // 512):
                        sl = slice(s * 512, (s + 1) * 512)
                        bp = pp.tile([32, 512], FP, tag="bp", bufs=2)
                        nc.tensor.matmul(bp[:], wb2_sb[:], fx[:, sl], start=True, stop=True)
                        nc.scalar.activation(cat[32:64, sl], bp[:], ACT_T.Relu, bias=bb2_sb[:])
                    e_t = cp2.tile([64, CH], BF, tag="e")
                    SL = 1024
                    for s in range(CH // SL):
                        lp = pp.tile([64, SL], FP, tag="lp")
                        for ss in range(SL // 512):
                            sl = slice(s * SL + ss * 512, s * SL + (ss + 1) * 512)
                            pslab = lp[:, ss * 512:(ss + 1) * 512]
                            nc.tensor.matmul(pslab, fc2_sb[:, :], cat[:, sl],
                                             start=True, stop=True)
                        nc.scalar.activation(e_t[:, s * SL:(s + 1) * SL], lp[:], ACT_T.Exp)
                    mt = cp2.tile([64, CH], BF, tag="mt", bufs=1)
                    nc.vector.tensor_tensor(mt[:], cat[:], e_t[:], ALU.mult)
                    agg = cp.tile([64, PTS], FP, tag="agg")
                    nc.vector.tensor_reduce(agg[:], mt[:].rearrange("p (n k) -> p n k", k=K),
                                            axis=AX.X, op=ALU.add)
                    se = cp.tile([64, PTS], FP, tag="se")
                    nc.vector.tensor_reduce(se[:], e_t[:].rearrange("p (n k) -> p n k", k=K),
                                            axis=AX.X, op=ALU.add)
                    rse = cp.tile([64, PTS], FP, tag="rse")
                    nc.vector.reciprocal(rse[:], se[:])
                    aggn = cp.tile([64, PTS], BF, tag="aggn")
                    nc.vector.tensor_tensor(aggn[:], agg[:], rse[:], ALU.mult)
                    # att2 mlp: wa2T^T @ aggn + ba2, relu -> f_agg2 [64, PTS]
                    fps = pp.tile([64, PTS], FP, tag="fps", bufs=1)
                    nc.tensor.matmul(fps[:], wa2_sb[:], aggn[:], start=True, stop=True)
                    # stacked rhs [97, PTS]: rows 0:64 f_agg2, 64:97 feat33_loc
                    cmb = cp.tile([97, PTS], BF, tag="cmb")
                    nc.scalar.activation(cmb[0:64, :], fps[:], ACT_T.Relu, bias=ba2_sb[:])
                    fl = cp.tile([33, PTS], FP, tag="fl")
                    nc.sync.dma_start(fl[:], feat33[:, p0:p0 + PTS])
                    nc.vector.tensor_copy(cmb[64:97, :], fl[:])
                    ops_ = pp.tile([128, PTS], FP, tag="ops", bufs=1)
                    nc.tensor.matmul(ops_[:], wmsc_sb[:], cmb[:], start=True, stop=True)
                    # leaky relu on DVE: max(x, 0.2x)
                    oo2 = cp.tile([128, PTS], FP, tag="oo2")
                    nc.vector.tensor_scalar(oo2[:], ops_[:], 0.2, None, ALU.mult)
                    oo = cp.tile([128, PTS], FP, tag="oo")
                    nc.vector.tensor_tensor(oo[:], ops_[:], oo2[:], ALU.max)
                    nc.sync.dma_start(out_o[:, p0:p0 + PTS], oo[:])
    nc.finalize()
    return nc


_EPS = 1e-5
N, NL, K = 32768, 8192, 16
BF16 = ml_dtypes.bfloat16

last_exec_ns = [0, 0]


def _fold(w, g, b):
    s = (np.asarray(g, np.float32) / np.sqrt(np.float32(1.0 + _EPS)))
    return np.asarray(w, np.float32) * s[:, None], np.asarray(b, np.float32)


def kernel(feature, xyz, w_mlp1, g_mlp1, b_mlp1, bb_w1, bb_g1, bb_b1,
           att1_fc, att1_w, att1_g, att1_b, bb_w2, bb_g2, bb_b2,
           att2_fc, att2_w, att2_g, att2_b, w_mlp2, g_mlp2, b_mlp2,
           w_sc, g_sc, b_sc, neigh_idx):
    from concourse.bass_utils import run_bass_kernel_spmd

    feature = np.asarray(feature, np.float32)
    xyz = np.asarray(xyz, np.float32)
    idx = np.asarray(neigh_idx).astype(np.int64)
    trace = bool(int(os.environ.get("RANDLA_TRACE", "0")))

    W1f, B1 = _fold(w_mlp1, g_mlp1, b_mlp1)
    Wb1f, Bb1 = _fold(bb_w1, bb_g1, bb_b1)
    Wa1f, Ba1 = _fold(att1_w, att1_g, att1_b)
    Wb2f, Bb2 = _fold(bb_w2, bb_g2, bb_b2)
    Wa2f, Ba2 = _fold(att2_w, att2_g, att2_b)
    Wm2f, Bm2 = _fold(w_mlp2, g_mlp2, b_mlp2)
    Wscf, Bsc = _fold(w_sc, g_sc, b_sc)
    Bout = Bm2 + Bsc
    A = Wb1f[:, 1:4] + Wb1f[:, 4:7]
    C = -Wb1f[:, 1:4] + Wb1f[:, 7:10]
    wd = Wb1f[:, 0]
    fc1 = np.asarray(att1_fc, np.float32)
    fc2 = np.asarray(att2_fc, np.float32)

    w1aug = np.ascontiguousarray(np.concatenate([W1f.T, B1[None, :]], 0))
    uaug = np.concatenate([A.T, Bb1[None, :]], 0).astype(BF16)
    wscaug = np.ascontiguousarray(
        np.concatenate([Wscf.T, Bout[None, :]], 0)).astype(np.float32)

    # per-core inputs
    ins1, idxws, feats, xyzT1s = [], [], [], []
    for c in range(8):
        b, q = c // 4, c % 4
        featb = np.ascontiguousarray(feature[b, :, :, 0])            # [32, N]
        feat33 = np.concatenate([featb, np.ones((1, N), np.float32)], 0)
        xyzb = np.ascontiguousarray(xyz[b])                          # [N, 3]
        xyzT1 = np.concatenate(
            [xyzb.T, np.ones((1, N), np.float32)], 0).astype(BF16)
        flat = idx[b, q * NL:(q + 1) * NL].reshape(-1).astype(np.int16)
        idxw = np.ascontiguousarray(
            np.tile(flat.reshape(NL * K // 16, 16).T, (8, 1)))
        feats.append(feat33)
        xyzT1s.append(xyzT1)
        idxws.append(idxw)
        ins1.append({
            "feat33": feat33, "xyz_rows": xyzb, "xyzT1": xyzT1, "idxw": idxw,
            "w1aug": w1aug, "vT": np.ascontiguousarray(C.T).astype(BF16),
            "uaug": uaug, "wd": np.ascontiguousarray(wd[:, None]),
            "fc1T": np.ascontiguousarray(fc1.T).astype(BF16),
            "wa1T": np.ascontiguousarray(Wa1f.T).astype(BF16),
            "ba1": np.ascontiguousarray(Ba1[:, None]),
        })

    nc1 = build_launch1()
    r1 = run_bass_kernel_spmd(nc1, ins1, core_ids=list(range(8)), trace=trace)
    last_exec_ns[0] = r1.exec_time_ns or 0

    fagg_full = [
        np.concatenate([r1.results[b * 4 + q]["f_agg"] for q in range(4)], 1)
        for b in range(2)
    ]  # [32, N] per batch

    ins2 = []
    for c in range(8):
        b, q = c // 4, c % 4
        ins2.append({
            "faggT": fagg_full[b], "fxyz": r1.results[c]["fxyz"],
            "feat33l": np.ascontiguousarray(feats[c][:, q * NL:(q + 1) * NL]),
            "idxw": idxws[c],
            "wb2T": np.ascontiguousarray(Wb2f.T).astype(BF16),
            "bb2": np.ascontiguousarray(Bb2[:, None]),
            "fc2T": np.ascontiguousarray(fc2.T).astype(BF16),
            "wa2T": np.ascontiguousarray(Wa2f.T).astype(BF16),
            "ba2": np.ascontiguousarray(Ba2[:, None]),
            "wm2T": np.ascontiguousarray(Wm2f.T).astype(BF16),
            "wscaug": wscaug,
        })

    nc2 = build_launch2()
    r2 = run_bass_kernel_spmd(nc2, ins2, core_ids=list(range(8)), trace=trace)
    last_exec_ns[1] = r2.exec_time_ns or 0

    out = np.empty((2, 128, N, 1), np.float32)
    for c in range(8):
        b, q = c // 4, c % 4
        out[b, :, q * NL:(q + 1) * NL, 0] = r2.results[c]["out"]
    return out



# revision 17
# speedup vs baseline: 2.7008x; 2.7008x over previous
"""Trainium Bass kernel for nn_Network_44968307589213 (RandLA-Net style
dilated residual block), SPMD across 8 NeuronCores.

Sharding: batch (2) x point-quarter (4): core c handles batch c//4,
points [(c%4)*8192, (c%4+1)*8192). Two NEFF launches with a host concat
of the per-quarter f_agg1 between them.

v2 design (vs baseline):
- Neighbor gathers run from an SBUF-resident table (dma_gather SBUF-source
  mode, tokens_per_rank=128) -> no HBM random traffic, no cross-core HBM
  contention (445us vs 989us per stage measured).
- All gpsimd partition_all_reduce / partition_broadcast ops eliminated.
- Rel-pos encoding: dist computed in a full-128-partition "spread" layout
  (DRAM bounce of the gathered nx channels), then DMA'd back as channel
  rows so ONE k=8 matmul produces the whole f_xyz pre-activation:
    pre = [C | b | wd | A] @ [nx(3); 1; dist; xtile(3)]
- Attention K-reduction via pairwise tree adds (2x DVE mode) instead of
  tensor_reduce (1x).
"""
import os
import numpy as np
import ml_dtypes

import concourse.bass as bass
import concourse.mybir as mybir
import concourse.tile as tile
from concourse import bacc

FP = mybir.dt.float32
BF = mybir.dt.bfloat16
I16 = mybir.dt.int16
AX = mybir.AxisListType
ALU = mybir.AluOpType
ACT_T = mybir.ActivationFunctionType

_EPS = 1e-5
N, NL, K = 32768, 8192, 16
E = NL * K            # 131072 edges per core
CH = 4096             # edges per chunk
NCH = E // CH         # 32 chunks
PTS = CH // K         # 256 points per chunk
MAC = 4               # chunks per spread-macro
NMAC = NCH // MAC     # 8 macros
EM = MAC * CH         # 16384 edges per macro
SPC = EM // 128       # 128 spread cols per macro
PPM = EM // K         # 1024 points per macro
BF16 = ml_dtypes.bfloat16

last_exec_ns = [0, 0]


def _att_tree(nc, cs, e_t, mt):
    """Pairwise-tree K-reduction of cat*e and e -> (agg, se, rse, aggn).

    r-halves live in rows 0:64, s-halves in rows 64:128 of shared tiles.
    """
    mtv = mt[:].rearrange("p (n k) -> p n k", k=16)
    etv = e_t[:].rearrange("p (n k) -> p n k", k=16)
    t1_ = cs.tile([128, PTS, 8], BF, tag="t1")
    nc.vector.tensor_tensor(t1_[0:64], mtv[:, :, 0:8], mtv[:, :, 8:16], ALU.add)
    nc.vector.tensor_tensor(t1_[64:128], etv[:, :, 0:8], etv[:, :, 8:16], ALU.add)
    t2_ = cs.tile([128, PTS, 4], BF, tag="t2")
    nc.vector.tensor_tensor(t2_[:], t1_[:, :, 0:4], t1_[:, :, 4:8], ALU.add)
    t3_ = cs.tile([128, PTS, 2], FP, tag="t3")
    nc.vector.tensor_tensor(t3_[:], t2_[:, :, 0:2], t2_[:, :, 2:4], ALU.add)
    t4_ = cs.tile([128, PTS], FP, tag="t4")
    nc.vector.tensor_tensor(t4_[:], t3_[:, :, 0], t3_[:, :, 1], ALU.add)
    rse = cs.tile([64, PTS], FP, tag="rse")
    nc.vector.reciprocal(rse[:], t4_[64:128])
    aggn = cs.tile([64, PTS], BF, tag="aggn")
    nc.vector.tensor_tensor(aggn[:], t4_[0:64], rse[:], ALU.mult)
    return aggn


GSUB = 1024  # idxs per dma_gather call: 66 descs/engine (~256-desc ring
             # holds several), so drains overlap across the 4 queues


def _gather(nc, G, tbl, idx_sb, e0, n, queue):
    for s0 in range(0, n, GSUB):
        nc.gpsimd.dma_gather(
            G[:, s0:s0 + GSUB].unsqueeze(1), tbl[:],
            idx_sb[:, (e0 + s0) // 16:(e0 + s0 + GSUB) // 16],
            num_idxs=GSUB, num_idxs_reg=GSUB, elem_size=128,
            transpose=True, single_packet=False,
            queue_num=(queue + s0 // GSUB) % 4,
            sbuf_tokens_per_rank=128,
            sbuf_free_dim_per_rank=256,
            sbuf_free_dim_pad_per_rank=0,
            sbuf_byte_offset=0)


def build_launch1(NQ=4, debug=False):
    nc = bacc.Bacc(name="rl1", num_swdge_queues=NQ)
    feat33 = nc.dram_tensor("feat33", [33, N], BF, kind="ExternalInput")
    xyzw = nc.dram_tensor("xyzw", [128, N // 128, 3], BF, kind="ExternalInput")
    xyzq = nc.dram_tensor("xyzq", [NL, 3], BF, kind="ExternalInput")
    idxw = nc.dram_tensor("idxw", [128, E // 16], I16, kind="ExternalInput")
    w1aug = nc.dram_tensor("w1aug", [33, 32], BF, kind="ExternalInput")
    combo8 = nc.dram_tensor("combo8", [8, 32], BF, kind="ExternalInput")
    fc1T = nc.dram_tensor("fc1T", [64, 64], BF, kind="ExternalInput")
    wa1T = nc.dram_tensor("wa1T", [64, 32], BF, kind="ExternalInput")
    ba1 = nc.dram_tensor("ba1", [32, 1], FP, kind="ExternalInput")

    f_agg_o = nc.dram_tensor("f_agg", [32, NL], FP, kind="ExternalOutput")
    fxyz_o = nc.dram_tensor("fxyz", [32, E], BF, kind="ExternalOutput")

    kind = dict(kind="ExternalOutput") if debug else {}
    nxd = nc.dram_tensor("nxd", [3, E], BF, **kind)  # gathered-nx bounce (up)
    dd = nc.dram_tensor("dd", [4, E], BF, **kind)    # dist+xtile pack (down)

    with tile.TileContext(nc) as tc:
        with tc.tile_pool(name="w", bufs=1) as wp:
            w1_sb = wp.tile([33, 32], BF)
            nc.sync.dma_start(w1_sb[:], w1aug[:])
            # combo8 staged at partitions 32:40 (matmul requires lhsT and
            # rhs to share the same base partition; rhs is G[32:40])
            cmb8_sb = wp.tile([40, 32], BF)
            nc.sync.dma_start(cmb8_sb[32:40, :], combo8[:])
            fc1_sb = wp.tile([64, 64], BF)
            nc.sync.dma_start(fc1_sb[:], fc1T[:])
            wa1_sb = wp.tile([64, 32], BF)
            nc.sync.dma_start(wa1_sb[:], wa1T[:])
            ba1_sb = wp.tile([32, 1], FP)
            nc.sync.dma_start(ba1_sb[:], ba1[:])
            idx_sb = wp.tile([128, E // 16], I16)
            nc.sync.dma_start(idx_sb[:], idxw[:])
            # SBUF gather table: point i -> partition i%128, rank i//128.
            # channels: 0:32 f_pc | 32:35 xyz | 35 ones | rest garbage
            tbl = wp.tile([128, N // 128, 128], BF)
            nc.vector.memset(tbl[:, :, 35:36], 1.0)
            xyzg = wp.tile([128, N // 128, 3], BF)
            nc.sync.dma_start(xyzg[:], xyzw[:])
            nc.vector.tensor_copy(tbl[:, :, 32:35], xyzg[:])

            with tc.tile_pool(name="pa", bufs=2, space="PSUM") as pa, \
                 tc.tile_pool(name="sa", bufs=3) as sa:
                TS = 2048
                for i in range(N // TS):
                    ft = sa.tile([33, TS], BF, tag="ft")
                    nc.sync.dma_start(ft[:], feat33[:, i * TS:(i + 1) * TS])
                    fpc = sa.tile([32, TS], BF, tag="fpc")
                    for s in range(TS // 1024):
                        ps = pa.tile([32, 1024], FP, tag="ps")
                        for ss in range(2):
                            sl = slice(s * 1024 + ss * 512, s * 1024 + (ss + 1) * 512)
                            nc.tensor.matmul(ps[:, ss * 512:(ss + 1) * 512],
                                             w1_sb[:], ft[:, sl],
                                             start=True, stop=True)
                        nc.scalar.activation(fpc[:, s * 1024:(s + 1) * 1024],
                                             ps[:], ACT_T.Relu)
                    tr = sa.tile([128, TS // 128, 32], BF, tag="tr")
                    nc.sync.dma_start_transpose(tr[:], fpc[:])
                    nc.vector.tensor_copy(
                        tbl[:, i * (TS // 128):(i + 1) * (TS // 128), 0:32], tr[:])

            # gather source/idx reads are invisible to the tile dep
            # tracker: make gpsimd read them once so every later gather
            # (same in-order engine) runs after the table/idx writes.
            guard = wp.tile([128, 256, 5], BF)
            nc.gpsimd.tensor_copy(guard[:], tbl[:, :, 31:36])
            guard2 = wp.tile([128, 1], I16)
            nc.gpsimd.tensor_copy(guard2[:], idx_sb[:, E // 16 - 1:])

            # ---------- chunk loop ----------
            with tc.tile_pool(name="cg", bufs=6) as cg, \
                 tc.tile_pool(name="ce", bufs=2) as ce, \
                 tc.tile_pool(name="cs", bufs=2) as cs, \
                 tc.tile_pool(name="sp", bufs=2) as sp, \
                 tc.tile_pool(name="p1", bufs=2, space="PSUM") as p1, \
                 tc.tile_pool(name="p2", bufs=1, space="PSUM") as p2, \
                 tc.tile_pool(name="p3", bufs=2, space="PSUM") as p3:
                Gs = []
                for m in range(NMAC):
                    # --- gathers + nx bounce-up for this macro ---
                    for tc_ in range(MAC):
                        t = m * MAC + tc_
                        e0 = t * CH
                        G = cg.tile([128, CH], BF, tag="G")
                        _gather(nc, G, tbl, idx_sb, e0, CH, t % NQ)
                        nc.sync.dma_start(nxd[:, e0:e0 + CH], G[32:35, :])
                        Gs.append(G)
                    em0 = m * EM
                    n0 = m * PPM
                    # --- spread: dist for the macro's 16384 edges ---
                    xyzr = sp.tile([128, PPM // 128, 3], BF, tag="xyzr")
                    nc.sync.dma_start(
                        xyzr[:],
                        xyzq[n0:n0 + PPM].rearrange("(p e) c -> p e c", p=128))
                    xt_sp = sp.tile([128, 3, SPC], BF, tag="xt_sp")
                    nx_sp = sp.tile([128, 3, SPC], BF, tag="nx_sp")
                    rel = sp.tile([128, 3, SPC], BF, tag="rel")
                    for d in range(3):
                        nc.vector.tensor_copy(
                            xt_sp[:, d, :].rearrange("p (e k) -> p e k", k=K),
                            xyzr[:, :, d].unsqueeze(2).broadcast_to(
                                [128, PPM // 128, K]))
                        nc.sync.dma_start(
                            nx_sp[:, d, :],
                            nxd[d, em0:em0 + EM].rearrange("(p f) -> p f", p=128))
                        nc.vector.tensor_tensor(rel[:, d, :], xt_sp[:, d, :],
                                                nx_sp[:, d, :], ALU.subtract)
                    d2 = sp.tile([128, SPC], FP, tag="d2")
                    sq = sp.tile([128, SPC], FP, tag="sq")
                    nc.vector.tensor_tensor(d2[:], rel[:, 0, :], rel[:, 0, :],
                                            ALU.mult)
                    nc.vector.tensor_tensor(sq[:], rel[:, 1, :], rel[:, 1, :],
                                            ALU.mult)
                    nc.vector.tensor_tensor(d2[:], d2[:], sq[:], ALU.add)
                    nc.vector.tensor_tensor(sq[:], rel[:, 2, :], rel[:, 2, :],
                                            ALU.mult)
                    nc.vector.tensor_tensor(d2[:], d2[:], sq[:], ALU.add)
                    dsp = sp.tile([128, SPC], BF, tag="dsp")
                    nc.scalar.activation(dsp[:], d2[:], ACT_T.Sqrt)
                    nc.sync.dma_start(
                        dd[0, em0:em0 + EM].rearrange("(p f) -> p f", p=128), dsp[:])
                    for d in range(3):
                        nc.sync.dma_start(
                            dd[1 + d, em0:em0 + EM].rearrange("(p f) -> p f", p=128),
                            xt_sp[:, d, :])

                    # --- per-chunk compute ---
                    for tc_ in range(MAC):
                        t = m * MAC + tc_
                        e0 = t * CH
                        p0 = t * PTS
                        G = Gs[t]
                        # scratch rows 36:40 <- [dist; xtile]
                        nc.sync.dma_start(G[36:40, :], dd[:, e0:e0 + CH])
                        # f_xyz = relu(combo8^T @ G[32:40])
                        for s in range(CH // 1024):
                            psx = p1.tile([32, 1024], FP, tag="psx")
                            for ss in range(2):
                                sl = slice(s * 1024 + ss * 512,
                                           s * 1024 + (ss + 1) * 512)
                                nc.tensor.matmul(psx[:, ss * 512:(ss + 1) * 512],
                                                 cmb8_sb[32:40, :], G[32:40, sl],
                                                 start=True, stop=True)
                            nc.scalar.activation(
                                G[32:64, s * 1024:(s + 1) * 1024], psx[:],
                                ACT_T.Relu)
                        # logits + exp
                        e_t = ce.tile([64, CH], BF, tag="e")
                        for s in range(CH // 1024):
                            psl = p2.tile([64, 1024], FP, tag="psl")
                            for ss in range(2):
                                sl = slice(s * 1024 + ss * 512,
                                           s * 1024 + (ss + 1) * 512)
                                nc.tensor.matmul(psl[:, ss * 512:(ss + 1) * 512],
                                                 fc1_sb[:], G[0:64, sl],
                                                 start=True, stop=True)
                            nc.scalar.activation(
                                e_t[:, s * 1024:(s + 1) * 1024], psl[:], ACT_T.Exp)
                        # weighted sums via pairwise tree over K
                        mt = ce.tile([64, CH], BF, tag="mt")
                        nc.vector.tensor_tensor(mt[:], G[0:64, :], e_t[:], ALU.mult)
                        # spill f_xyz
                        nc.sync.dma_start(fxyz_o[:, e0:e0 + CH], G[32:64, :])
                        aggn = _att_tree(nc, cs, e_t, mt)
                        # att1 mlp
                        fps = p3.tile([32, PTS], FP, tag="fps")
                        nc.tensor.matmul(fps[:], wa1_sb[:], aggn[:],
                                         start=True, stop=True)
                        fago = cs.tile([32, PTS], FP, tag="fago")
                        nc.scalar.activation(fago[:], fps[:], ACT_T.Relu,
                                             bias=ba1_sb[:])
                        nc.sync.dma_start(f_agg_o[:, p0:p0 + PTS], fago[:])
    nc.finalize()
    return nc


def build_launch2(NQ=4):
    nc = bacc.Bacc(name="rl2", num_swdge_queues=NQ)
    faggT = nc.dram_tensor("faggT", [32, N], BF, kind="ExternalInput")
    fxyz_i = nc.dram_tensor("fxyz", [32, E], BF, kind="ExternalInput")
    feat33l = nc.dram_tensor("feat33l", [33, NL], BF, kind="ExternalInput")
    idxw = nc.dram_tensor("idxw", [128, E // 16], I16, kind="ExternalInput")
    wb2T = nc.dram_tensor("wb2T", [32, 32], BF, kind="ExternalInput")
    bb2 = nc.dram_tensor("bb2", [32, 1], FP, kind="ExternalInput")
    fc2T = nc.dram_tensor("fc2T", [64, 64], BF, kind="ExternalInput")
    wa2T = nc.dram_tensor("wa2T", [64, 64], BF, kind="ExternalInput")
    ba2 = nc.dram_tensor("ba2", [64, 1], FP, kind="ExternalInput")
    wmscT = nc.dram_tensor("wmscT", [97, 128], BF, kind="ExternalInput")

    out_o = nc.dram_tensor("out", [128, NL], FP, kind="ExternalOutput")

    with tile.TileContext(nc) as tc:
        with tc.tile_pool(name="w", bufs=1) as wp:
            wb2_sb = wp.tile([32, 32], BF)
            nc.sync.dma_start(wb2_sb[:], wb2T[:])
            bb2_sb = wp.tile([32, 1], FP)
            nc.sync.dma_start(bb2_sb[:], bb2[:])
            fc2_sb = wp.tile([64, 64], BF)
            nc.sync.dma_start(fc2_sb[:], fc2T[:])
            wa2_sb = wp.tile([64, 64], BF)
            nc.sync.dma_start(wa2_sb[:], wa2T[:])
            ba2_sb = wp.tile([64, 1], FP)
            nc.sync.dma_start(ba2_sb[:], ba2[:])
            wmsc_sb = wp.tile([97, 128], BF)
            nc.sync.dma_start(wmsc_sb[:], wmscT[:])
            idx_sb = wp.tile([128, E // 16], I16)
            nc.sync.dma_start(idx_sb[:], idxw[:])
            tbl = wp.tile([128, N // 128, 128], BF)

            with tc.tile_pool(name="sa", bufs=3) as sa:
                TS = 2048
                for i in range(N // TS):
                    fa = sa.tile([32, TS], BF, tag="fa")
                    nc.sync.dma_start(fa[:], faggT[:, i * TS:(i + 1) * TS])
                    tr = sa.tile([128, TS // 128, 32], BF, tag="tr")
                    nc.sync.dma_start_transpose(tr[:], fa[:])
                    nc.vector.tensor_copy(
                        tbl[:, i * (TS // 128):(i + 1) * (TS // 128), 0:32], tr[:])

            with tc.tile_pool(name="cg", bufs=6) as cg, \
                 tc.tile_pool(name="ce", bufs=2) as ce, \
                 tc.tile_pool(name="cs", bufs=2) as cs, \
                 tc.tile_pool(name="p1", bufs=2, space="PSUM") as p1, \
                 tc.tile_pool(name="p2", bufs=1, space="PSUM") as p2, \
                 tc.tile_pool(name="p3", bufs=1, space="PSUM") as p3:
                for t in range(NCH):
                    e0 = t * CH
                    p0 = t * PTS
                    G = cg.tile([128, CH], BF, tag="G")
                    _gather(nc, G, tbl, idx_sb, e0, CH, t % 4)
                    fx = cg.tile([32, CH], BF, tag="fx", bufs=2)
                    nc.sync.dma_start(fx[:], fxyz_i[:, e0:e0 + CH])
                    # f_xyz2 = relu(wb2 @ fx + bb2) -> G[32:64]
                    for s in range(CH // 1024):
                        psx = p1.tile([32, 1024], FP, tag="psx")
                        for ss in range(2):
                            sl = slice(s * 1024 + ss * 512,
                                       s * 1024 + (ss + 1) * 512)
                            nc.tensor.matmul(psx[:, ss * 512:(ss + 1) * 512],
                                             wb2_sb[:], fx[:, sl],
                                             start=True, stop=True)
                        nc.scalar.activation(G[32:64, s * 1024:(s + 1) * 1024],
                                             psx[:], ACT_T.Relu, bias=bb2_sb[:])
                    e_t = ce.tile([64, CH], BF, tag="e")
                    for s in range(CH // 1024):
                        psl = p2.tile([64, 1024], FP, tag="psl")
                        for ss in range(2):
                            sl = slice(s * 1024 + ss * 512,
                                       s * 1024 + (ss + 1) * 512)
                            nc.tensor.matmul(psl[:, ss * 512:(ss + 1) * 512],
                                             fc2_sb[:], G[0:64, sl],
                                             start=True, stop=True)
                        nc.scalar.activation(e_t[:, s * 1024:(s + 1) * 1024],
                                             psl[:], ACT_T.Exp)
                    mt = ce.tile([64, CH], BF, tag="mt")
                    nc.vector.tensor_tensor(mt[:], G[0:64, :], e_t[:], ALU.mult)
                    aggn = _att_tree(nc, cs, e_t, mt)
                    # att2 mlp (64 out) + stacked final matmul
                    fps = p3.tile([64, PTS], FP, tag="fps")
                    nc.tensor.matmul(fps[:], wa2_sb[:], aggn[:], start=True, stop=True)
                    cmb = cs.tile([97, PTS], BF, tag="cmb")
                    nc.scalar.activation(cmb[0:64, :], fps[:], ACT_T.Relu,
                                         bias=ba2_sb[:])
                    nc.sync.dma_start(cmb[64:97, :], feat33l[:, p0:p0 + PTS])
                    pf = p3.tile([128, PTS], FP, tag="pf")
                    nc.tensor.matmul(pf[:], wmsc_sb[:], cmb[:], start=True, stop=True)
                    # leaky relu (0.2): Lrelu activation ignores alpha (applies
                    # 0.01), so do max(x, 0.2x) explicitly
                    oo2 = cs.tile([128, PTS], FP, tag="oo2")
                    nc.vector.tensor_scalar(oo2[:], pf[:], 0.2, None, ALU.mult)
                    oo = cs.tile([128, PTS], FP, tag="oo")
                    nc.vector.tensor_tensor(oo[:], pf[:], oo2[:], ALU.max)
                    nc.sync.dma_start(out_o[:, p0:p0 + PTS], oo[:])
    nc.finalize()
    return nc


def _fold(w, g, b):
    s = (np.asarray(g, np.float32) / np.sqrt(np.float32(1.0 + _EPS)))
    return np.asarray(w, np.float32) * s[:, None], np.asarray(b, np.float32)


def kernel(feature, xyz, w_mlp1, g_mlp1, b_mlp1, bb_w1, bb_g1, bb_b1,
           att1_fc, att1_w, att1_g, att1_b, bb_w2, bb_g2, bb_b2,
           att2_fc, att2_w, att2_g, att2_b, w_mlp2, g_mlp2, b_mlp2,
           w_sc, g_sc, b_sc, neigh_idx):
    from concourse.bass_utils import run_bass_kernel_spmd

    feature = np.asarray(feature, np.float32)
    xyz = np.asarray(xyz, np.float32)
    idx = np.asarray(neigh_idx).astype(np.int64)
    trace = bool(int(os.environ.get("RANDLA_TRACE", "0")))

    W1f, B1 = _fold(w_mlp1, g_mlp1, b_mlp1)
    Wb1f, Bb1 = _fold(bb_w1, bb_g1, bb_b1)
    Wa1f, Ba1 = _fold(att1_w, att1_g, att1_b)
    Wb2f, Bb2 = _fold(bb_w2, bb_g2, bb_b2)
    Wa2f, Ba2 = _fold(att2_w, att2_g, att2_b)
    Wm2f, Bm2 = _fold(w_mlp2, g_mlp2, b_mlp2)
    Wscf, Bsc = _fold(w_sc, g_sc, b_sc)
    Bout = Bm2 + Bsc
    A = Wb1f[:, 1:4] + Wb1f[:, 4:7]
    C = -Wb1f[:, 1:4] + Wb1f[:, 7:10]
    wd = Wb1f[:, 0]
    fc1 = np.asarray(att1_fc, np.float32)
    fc2 = np.asarray(att2_fc, np.float32)

    w1aug = np.concatenate([W1f.T, B1[None, :]], 0).astype(BF16)
    # combo8 rows must match rhs rows [nx(3); ones; dist; xtile(3)]
    combo8 = np.concatenate(
        [C.T, Bb1[None, :], wd[None, :], A.T], 0).astype(BF16)
    wmscT = np.concatenate([Wm2f.T, Wscf.T, Bout[None, :]], 0).astype(BF16)

    ins1, idxws, feats = [], [], []
    for c in range(8):
        b, q = c // 4, c % 4
        featb = np.ascontiguousarray(feature[b, :, :, 0])            # [32, N]
        feat33 = np.concatenate([featb, np.ones((1, N), np.float32)], 0)
        xyzb = np.ascontiguousarray(xyz[b]).astype(BF16)             # [N, 3]
        flat = idx[b, q * NL:(q + 1) * NL].reshape(-1).astype(np.int16)
        idxw = np.ascontiguousarray(
            np.tile(flat.reshape(E // 16, 16).T, (8, 1)))
        feats.append(feat33)
        idxws.append(idxw)
        xyzw = np.ascontiguousarray(
            xyzb.reshape(N // 128, 128, 3).transpose(1, 0, 2))
        ins1.append({
            "feat33": feat33.astype(BF16), "xyzw": xyzw,
            "xyzq": np.ascontiguousarray(xyzb[q * NL:(q + 1) * NL]), "idxw": idxw,
            "w1aug": w1aug, "combo8": combo8,
            "fc1T": np.ascontiguousarray(fc1.T).astype(BF16),
            "wa1T": np.ascontiguousarray(Wa1f.T).astype(BF16),
            "ba1": np.ascontiguousarray(Ba1[:, None]),
        })

    nc1 = build_launch1()
    r1 = run_bass_kernel_spmd(nc1, ins1, core_ids=list(range(8)), trace=trace)
    last_exec_ns[0] = r1.exec_time_ns or 0

    fagg_full = [
        np.concatenate([r1.results[b * 4 + q]["f_agg"] for q in range(4)], 1)
        for b in range(2)
    ]  # [32, N] fp32 per batch

    ins2 = []
    for c in range(8):
        b, q = c // 4, c % 4
        ins2.append({
            "faggT": fagg_full[b].astype(BF16), "fxyz": r1.results[c]["fxyz"],
            "feat33l": np.ascontiguousarray(
                feats[c][:, q * NL:(q + 1) * NL]).astype(BF16),
            "idxw": idxws[c],
            "wb2T": np.ascontiguousarray(Wb2f.T).astype(BF16),
            "bb2": np.ascontiguousarray(Bb2[:, None]),
            "fc2T": np.ascontiguousarray(fc2.T).astype(BF16),
            "wa2T": np.ascontiguousarray(Wa2f.T).astype(BF16),
            "ba2": np.ascontiguousarray(Ba2[:, None]),
            "wmscT": wmscT,
        })

    nc2 = build_launch2()
    r2 = run_bass_kernel_spmd(nc2, ins2, core_ids=list(range(8)), trace=trace)
    last_exec_ns[1] = r2.exec_time_ns or 0

    out = np.empty((2, 128, N, 1), np.float32)
    for c in range(8):
        b, q = c // 4, c % 4
        out[b, :, q * NL:(q + 1) * NL, 0] = r2.results[c]["out"]
    return out


# revision 19
# speedup vs baseline: 2.8836x; 1.0676x over previous
"""Trainium Bass kernel for nn_Network_44968307589213 (RandLA-Net style
dilated residual block), SPMD across 8 NeuronCores.

Sharding: batch (2) x point-quarter (4): core c handles batch c//4,
points [(c%4)*8192, (c%4+1)*8192). Two NEFF launches with a host concat
of the per-quarter f_agg1 between them.

v2 design (vs baseline):
- Neighbor gathers run from an SBUF-resident table (dma_gather SBUF-source
  mode, tokens_per_rank=128) -> no HBM random traffic, no cross-core HBM
  contention (445us vs 989us per stage measured).
- All gpsimd partition_all_reduce / partition_broadcast ops eliminated.
- Rel-pos encoding: dist computed in a full-128-partition "spread" layout
  (DRAM bounce of the gathered nx channels), then DMA'd back as channel
  rows so ONE k=8 matmul produces the whole f_xyz pre-activation:
    pre = [C | b | wd | A] @ [nx(3); 1; dist; xtile(3)]
- Attention K-reduction via pairwise tree adds (2x DVE mode) instead of
  tensor_reduce (1x).
"""
import os
import numpy as np
import ml_dtypes

import concourse.bass as bass
import concourse.mybir as mybir
import concourse.tile as tile
from concourse import bacc

FP = mybir.dt.float32
BF = mybir.dt.bfloat16
I16 = mybir.dt.int16
AX = mybir.AxisListType
ALU = mybir.AluOpType
ACT_T = mybir.ActivationFunctionType

_EPS = 1e-5
N, NL, K = 32768, 8192, 16
E = NL * K            # 131072 edges per core
CH = 4096             # edges per chunk
NCH = E // CH         # 32 chunks
PTS = CH // K         # 256 points per chunk
MAC = 4               # chunks per spread-macro
NMAC = NCH // MAC     # 8 macros
EM = MAC * CH         # 16384 edges per macro
SPC = EM // 128       # 128 spread cols per macro
PPM = EM // K         # 1024 points per macro
BF16 = ml_dtypes.bfloat16

last_exec_ns = [0, 0]


def _att_tree(nc, cs, e_t, mt):
    """Pairwise-tree K-reduction of cat*e and e -> (agg, se, rse, aggn).

    r-halves live in rows 0:64, s-halves in rows 64:128 of shared tiles.
    """
    mtv = mt[:].rearrange("p (n k) -> p n k", k=16)
    etv = e_t[:].rearrange("p (n k) -> p n k", k=16)
    t1_ = cs.tile([128, PTS, 8], BF, tag="t1")
    nc.vector.tensor_tensor(t1_[0:64], mtv[:, :, 0:8], mtv[:, :, 8:16], ALU.add)
    nc.vector.tensor_tensor(t1_[64:128], etv[:, :, 0:8], etv[:, :, 8:16], ALU.add)
    t2_ = cs.tile([128, PTS, 4], BF, tag="t2")
    nc.vector.tensor_tensor(t2_[:], t1_[:, :, 0:4], t1_[:, :, 4:8], ALU.add)
    t3_ = cs.tile([128, PTS, 2], FP, tag="t3")
    nc.vector.tensor_tensor(t3_[:], t2_[:, :, 0:2], t2_[:, :, 2:4], ALU.add)
    t4_ = cs.tile([128, PTS], FP, tag="t4")
    nc.vector.tensor_tensor(t4_[:], t3_[:, :, 0], t3_[:, :, 1], ALU.add)
    rse = cs.tile([64, PTS], FP, tag="rse")
    nc.vector.reciprocal(rse[:], t4_[64:128])
    aggn = cs.tile([64, PTS], BF, tag="aggn")
    nc.vector.tensor_tensor(aggn[:], t4_[0:64], rse[:], ALU.mult)
    return aggn


GSUB = 1024  # idxs per dma_gather call: 66 descs/engine (~256-desc ring
             # holds several), so drains overlap across the 4 queues


def _gather(nc, G, tbl, idx_sb, e0, n, queue):
    for s0 in range(0, n, GSUB):
        nc.gpsimd.dma_gather(
            G[:, s0:s0 + GSUB].unsqueeze(1), tbl[:],
            idx_sb[:, (e0 + s0) // 16:(e0 + s0 + GSUB) // 16],
            num_idxs=GSUB, num_idxs_reg=GSUB, elem_size=128,
            transpose=True, single_packet=False,
            queue_num=(queue + s0 // GSUB) % 4,
            sbuf_tokens_per_rank=128,
            sbuf_free_dim_per_rank=256,
            sbuf_free_dim_pad_per_rank=0,
            sbuf_byte_offset=0)


def build_launch1(NQ=4, debug=False):
    nc = bacc.Bacc(name="rl1", num_swdge_queues=NQ)
    feat33 = nc.dram_tensor("feat33", [33, N], BF, kind="ExternalInput")
    xyzw = nc.dram_tensor("xyzw", [128, N // 128, 3], BF, kind="ExternalInput")
    xyzq = nc.dram_tensor("xyzq", [NL, 3], BF, kind="ExternalInput")
    idxw = nc.dram_tensor("idxw", [128, E // 16], I16, kind="ExternalInput")
    w1aug = nc.dram_tensor("w1aug", [33, 32], BF, kind="ExternalInput")
    combo8 = nc.dram_tensor("combo8", [8, 32], BF, kind="ExternalInput")
    fc1T = nc.dram_tensor("fc1T", [64, 64], BF, kind="ExternalInput")
    wa1T = nc.dram_tensor("wa1T", [64, 32], BF, kind="ExternalInput")
    ba1 = nc.dram_tensor("ba1", [32, 1], FP, kind="ExternalInput")

    f_agg_o = nc.dram_tensor("f_agg", [32, NL], FP, kind="ExternalOutput")
    fxyz_o = nc.dram_tensor("fxyz", [32, E], BF, kind="ExternalOutput")

    kind = dict(kind="ExternalOutput") if debug else {}
    nxd = nc.dram_tensor("nxd", [3, E], BF, **kind)  # gathered-nx bounce (up)
    dd = nc.dram_tensor("dd", [4, E], BF, **kind)    # dist+xtile pack (down)

    with tile.TileContext(nc) as tc:
        with tc.tile_pool(name="w", bufs=1) as wp:
            w1_sb = wp.tile([33, 32], BF)
            nc.sync.dma_start(w1_sb[:], w1aug[:])
            # combo8 staged at partitions 32:40 (matmul requires lhsT and
            # rhs to share the same base partition; rhs is G[32:40])
            cmb8_sb = wp.tile([40, 32], BF)
            nc.sync.dma_start(cmb8_sb[32:40, :], combo8[:])
            fc1_sb = wp.tile([64, 64], BF)
            nc.sync.dma_start(fc1_sb[:], fc1T[:])
            wa1_sb = wp.tile([64, 32], BF)
            nc.sync.dma_start(wa1_sb[:], wa1T[:])
            ba1_sb = wp.tile([32, 1], FP)
            nc.sync.dma_start(ba1_sb[:], ba1[:])
            idx_sb = wp.tile([128, E // 16], I16)
            nc.sync.dma_start(idx_sb[:], idxw[:])
            # SBUF gather table: point i -> partition i%128, rank i//128.
            # channels: 0:32 f_pc | 32:35 xyz | 35 ones | rest garbage
            tbl = wp.tile([128, N // 128, 128], BF)
            nc.vector.memset(tbl[:, :, 35:36], 1.0)
            xyzg = wp.tile([128, N // 128, 3], BF)
            nc.sync.dma_start(xyzg[:], xyzw[:])
            nc.vector.tensor_copy(tbl[:, :, 32:35], xyzg[:])

            with tc.tile_pool(name="pa", bufs=2, space="PSUM") as pa, \
                 tc.tile_pool(name="sa", bufs=3) as sa:
                TS = 2048
                for i in range(N // TS):
                    ft = sa.tile([33, TS], BF, tag="ft")
                    nc.sync.dma_start(ft[:], feat33[:, i * TS:(i + 1) * TS])
                    fpc = sa.tile([32, TS], BF, tag="fpc")
                    for s in range(TS // 1024):
                        ps = pa.tile([32, 1024], FP, tag="ps")
                        for ss in range(2):
                            sl = slice(s * 1024 + ss * 512, s * 1024 + (ss + 1) * 512)
                            nc.tensor.matmul(ps[:, ss * 512:(ss + 1) * 512],
                                             w1_sb[:], ft[:, sl],
                                             start=True, stop=True)
                        nc.scalar.activation(fpc[:, s * 1024:(s + 1) * 1024],
                                             ps[:], ACT_T.Relu)
                    tr = sa.tile([128, TS // 128, 32], BF, tag="tr")
                    nc.sync.dma_start_transpose(tr[:], fpc[:])
                    nc.vector.tensor_copy(
                        tbl[:, i * (TS // 128):(i + 1) * (TS // 128), 0:32], tr[:])

            # gather source/idx reads are invisible to the tile dep
            # tracker: make gpsimd read them once so every later gather
            # (same in-order engine) runs after the table/idx writes.
            guard = wp.tile([128, 256, 5], BF)
            nc.gpsimd.tensor_copy(guard[:], tbl[:, :, 31:36])
            guard2 = wp.tile([128, 1], I16)
            nc.gpsimd.tensor_copy(guard2[:], idx_sb[:, E // 16 - 1:])

            # ---------- chunk loop ----------
            with tc.tile_pool(name="cg", bufs=6) as cg, \
                 tc.tile_pool(name="ce", bufs=2) as ce, \
                 tc.tile_pool(name="cs", bufs=2) as cs, \
                 tc.tile_pool(name="sp", bufs=3) as sp, \
                 tc.tile_pool(name="p1", bufs=2, space="PSUM") as p1, \
                 tc.tile_pool(name="p2", bufs=1, space="PSUM") as p2, \
                 tc.tile_pool(name="p3", bufs=2, space="PSUM") as p3:
                Gs = []
                for m in range(NMAC):
                    # --- gathers + nx bounce-up for this macro ---
                    for tc_ in range(MAC):
                        t = m * MAC + tc_
                        e0 = t * CH
                        G = cg.tile([128, CH], BF, tag="G")
                        _gather(nc, G, tbl, idx_sb, e0, CH, t % NQ)
                        nc.sync.dma_start(nxd[:, e0:e0 + CH], G[32:35, :])
                        Gs.append(G)
                    em0 = m * EM
                    n0 = m * PPM
                    # --- spread: dist for the macro's 16384 edges ---
                    xyzr = sp.tile([128, PPM // 128, 3], BF, tag="xyzr")
                    nc.sync.dma_start(
                        xyzr[:],
                        xyzq[n0:n0 + PPM].rearrange("(p e) c -> p e c", p=128))
                    xt_sp = sp.tile([128, 3, SPC], BF, tag="xt_sp")
                    nx_sp = sp.tile([128, 3, SPC], BF, tag="nx_sp")
                    rel = sp.tile([128, 3, SPC], BF, tag="rel")
                    for d in range(3):
                        nc.vector.tensor_copy(
                            xt_sp[:, d, :].rearrange("p (e k) -> p e k", k=K),
                            xyzr[:, :, d].unsqueeze(2).broadcast_to(
                                [128, PPM // 128, K]))
                        nc.sync.dma_start(
                            nx_sp[:, d, :],
                            nxd[d, em0:em0 + EM].rearrange("(p f) -> p f", p=128))
                        nc.vector.tensor_tensor(rel[:, d, :], xt_sp[:, d, :],
                                                nx_sp[:, d, :], ALU.subtract)
                    d2 = sp.tile([128, SPC], FP, tag="d2")
                    sq = sp.tile([128, SPC], FP, tag="sq")
                    nc.vector.tensor_tensor(d2[:], rel[:, 0, :], rel[:, 0, :],
                                            ALU.mult)
                    nc.vector.tensor_tensor(sq[:], rel[:, 1, :], rel[:, 1, :],
                                            ALU.mult)
                    nc.vector.tensor_tensor(d2[:], d2[:], sq[:], ALU.add)
                    nc.vector.tensor_tensor(sq[:], rel[:, 2, :], rel[:, 2, :],
                                            ALU.mult)
                    nc.vector.tensor_tensor(d2[:], d2[:], sq[:], ALU.add)
                    dsp = sp.tile([128, SPC], BF, tag="dsp")
                    nc.scalar.activation(dsp[:], d2[:], ACT_T.Sqrt)
                    nc.sync.dma_start(
                        dd[0, em0:em0 + EM].rearrange("(p f) -> p f", p=128), dsp[:])
                    for d in range(3):
                        nc.sync.dma_start(
                            dd[1 + d, em0:em0 + EM].rearrange("(p f) -> p f", p=128),
                            xt_sp[:, d, :])

                    # --- per-chunk compute ---
                    for tc_ in range(MAC):
                        t = m * MAC + tc_
                        e0 = t * CH
                        p0 = t * PTS
                        G = Gs[t]
                        # scratch rows 36:40 <- [dist; xtile]
                        nc.sync.dma_start(G[36:40, :], dd[:, e0:e0 + CH])
                        # f_xyz = relu(combo8^T @ G[32:40])
                        for s in range(CH // 1024):
                            psx = p1.tile([32, 1024], FP, tag="psx")
                            for ss in range(2):
                                sl = slice(s * 1024 + ss * 512,
                                           s * 1024 + (ss + 1) * 512)
                                nc.tensor.matmul(psx[:, ss * 512:(ss + 1) * 512],
                                                 cmb8_sb[32:40, :], G[32:40, sl],
                                                 start=True, stop=True)
                            nc.scalar.activation(
                                G[32:64, s * 1024:(s + 1) * 1024], psx[:],
                                ACT_T.Relu)
                        # logits + exp
                        e_t = ce.tile([64, CH], BF, tag="e")
                        for s in range(CH // 1024):
                            psl = p2.tile([64, 1024], FP, tag="psl")
                            for ss in range(2):
                                sl = slice(s * 1024 + ss * 512,
                                           s * 1024 + (ss + 1) * 512)
                                nc.tensor.matmul(psl[:, ss * 512:(ss + 1) * 512],
                                                 fc1_sb[:], G[0:64, sl],
                                                 start=True, stop=True)
                            nc.scalar.activation(
                                e_t[:, s * 1024:(s + 1) * 1024], psl[:], ACT_T.Exp)
                        # weighted sums via pairwise tree over K
                        mt = ce.tile([64, CH], BF, tag="mt")
                        nc.vector.tensor_tensor(mt[:], G[0:64, :], e_t[:], ALU.mult)
                        # spill f_xyz
                        nc.sync.dma_start(fxyz_o[:, e0:e0 + CH], G[32:64, :])
                        aggn = _att_tree(nc, cs, e_t, mt)
                        # att1 mlp
                        fps = p3.tile([32, PTS], FP, tag="fps")
                        nc.tensor.matmul(fps[:], wa1_sb[:], aggn[:],
                                         start=True, stop=True)
                        fago = cs.tile([32, PTS], FP, tag="fago")
                        nc.scalar.activation(fago[:], fps[:], ACT_T.Relu,
                                             bias=ba1_sb[:])
                        nc.sync.dma_start(f_agg_o[:, p0:p0 + PTS], fago[:])
    nc.finalize()
    return nc


def build_launch2(NQ=4):
    nc = bacc.Bacc(name="rl2", num_swdge_queues=NQ)
    faggT = nc.dram_tensor("faggT", [32, N], BF, kind="ExternalInput")
    fxyz_i = nc.dram_tensor("fxyz", [32, E], BF, kind="ExternalInput")
    feat33l = nc.dram_tensor("feat33l", [33, NL], BF, kind="ExternalInput")
    idxw = nc.dram_tensor("idxw", [128, E // 16], I16, kind="ExternalInput")
    wb2T = nc.dram_tensor("wb2T", [32, 32], BF, kind="ExternalInput")
    bb2 = nc.dram_tensor("bb2", [32, 1], FP, kind="ExternalInput")
    fc2T = nc.dram_tensor("fc2T", [64, 64], BF, kind="ExternalInput")
    wa2T = nc.dram_tensor("wa2T", [64, 64], BF, kind="ExternalInput")
    ba2 = nc.dram_tensor("ba2", [64, 1], FP, kind="ExternalInput")
    wmscT = nc.dram_tensor("wmscT", [97, 128], BF, kind="ExternalInput")

    out_o = nc.dram_tensor("out", [128, NL], FP, kind="ExternalOutput")

    with tile.TileContext(nc) as tc:
        with tc.tile_pool(name="w", bufs=1) as wp:
            wb2_sb = wp.tile([32, 32], BF)
            nc.sync.dma_start(wb2_sb[:], wb2T[:])
            bb2_sb = wp.tile([32, 1], FP)
            nc.sync.dma_start(bb2_sb[:], bb2[:])
            fc2_sb = wp.tile([64, 64], BF)
            nc.sync.dma_start(fc2_sb[:], fc2T[:])
            wa2_sb = wp.tile([64, 64], BF)
            nc.sync.dma_start(wa2_sb[:], wa2T[:])
            ba2_sb = wp.tile([64, 1], FP)
            nc.sync.dma_start(ba2_sb[:], ba2[:])
            wmsc_sb = wp.tile([97, 128], BF)
            nc.sync.dma_start(wmsc_sb[:], wmscT[:])
            idx_sb = wp.tile([128, E // 16], I16)
            nc.sync.dma_start(idx_sb[:], idxw[:])
            tbl = wp.tile([128, N // 128, 128], BF)

            with tc.tile_pool(name="sa", bufs=3) as sa:
                TS = 2048
                for i in range(N // TS):
                    fa = sa.tile([32, TS], BF, tag="fa")
                    nc.sync.dma_start(fa[:], faggT[:, i * TS:(i + 1) * TS])
                    tr = sa.tile([128, TS // 128, 32], BF, tag="tr")
                    nc.sync.dma_start_transpose(tr[:], fa[:])
                    nc.vector.tensor_copy(
                        tbl[:, i * (TS // 128):(i + 1) * (TS // 128), 0:32], tr[:])

            with tc.tile_pool(name="cg", bufs=6) as cg, \
                 tc.tile_pool(name="ce", bufs=2) as ce, \
                 tc.tile_pool(name="cs", bufs=2) as cs, \
                 tc.tile_pool(name="p1", bufs=2, space="PSUM") as p1, \
                 tc.tile_pool(name="p2", bufs=1, space="PSUM") as p2, \
                 tc.tile_pool(name="p3", bufs=1, space="PSUM") as p3:
                for t in range(NCH):
                    e0 = t * CH
                    p0 = t * PTS
                    G = cg.tile([128, CH], BF, tag="G")
                    _gather(nc, G, tbl, idx_sb, e0, CH, t % 4)
                    fx = cg.tile([32, CH], BF, tag="fx", bufs=2)
                    nc.sync.dma_start(fx[:], fxyz_i[:, e0:e0 + CH])
                    # cat2 decoupled from G: relu (f_xyz2) does not wait for
                    # the gather drain; only the cheap copy does.
                    cat2 = ce.tile([64, CH], BF, tag="cat2")
                    nc.vector.tensor_copy(cat2[0:32, :], G[0:32, :])
                    for s in range(CH // 1024):
                        psx = p1.tile([32, 1024], FP, tag="psx")
                        for ss in range(2):
                            sl = slice(s * 1024 + ss * 512,
                                       s * 1024 + (ss + 1) * 512)
                            nc.tensor.matmul(psx[:, ss * 512:(ss + 1) * 512],
                                             wb2_sb[:], fx[:, sl],
                                             start=True, stop=True)
                        nc.scalar.activation(cat2[32:64, s * 1024:(s + 1) * 1024],
                                             psx[:], ACT_T.Relu, bias=bb2_sb[:])
                    e_t = ce.tile([64, CH], BF, tag="e")
                    for s in range(CH // 1024):
                        psl = p2.tile([64, 1024], FP, tag="psl")
                        for ss in range(2):
                            sl = slice(s * 1024 + ss * 512,
                                       s * 1024 + (ss + 1) * 512)
                            nc.tensor.matmul(psl[:, ss * 512:(ss + 1) * 512],
                                             fc2_sb[:], cat2[:, sl],
                                             start=True, stop=True)
                        nc.scalar.activation(e_t[:, s * 1024:(s + 1) * 1024],
                                             psl[:], ACT_T.Exp)
                    # mt in place: cat2 <- cat2 * e (cat2 not needed after)
                    nc.vector.tensor_tensor(cat2[:], cat2[:], e_t[:], ALU.mult)
                    aggn = _att_tree(nc, cs, e_t, cat2)
                    # att2 mlp (64 out) + stacked final matmul
                    fps = p3.tile([64, PTS], FP, tag="fps")
                    nc.tensor.matmul(fps[:], wa2_sb[:], aggn[:], start=True, stop=True)
                    cmb = cs.tile([97, PTS], BF, tag="cmb")
                    nc.scalar.activation(cmb[0:64, :], fps[:], ACT_T.Relu,
                                         bias=ba2_sb[:])
                    nc.sync.dma_start(cmb[64:97, :], feat33l[:, p0:p0 + PTS])
                    pf = p3.tile([128, PTS], FP, tag="pf")
                    nc.tensor.matmul(pf[:], wmsc_sb[:], cmb[:], start=True, stop=True)
                    # leaky relu (0.2): Lrelu activation ignores alpha (applies
                    # 0.01), so do max(x, 0.2x) explicitly
                    oo2 = cs.tile([128, PTS], FP, tag="oo2")
                    nc.vector.tensor_scalar(oo2[:], pf[:], 0.2, None, ALU.mult)
                    oo = cs.tile([128, PTS], FP, tag="oo")
                    nc.vector.tensor_tensor(oo[:], pf[:], oo2[:], ALU.max)
                    nc.sync.dma_start(out_o[:, p0:p0 + PTS], oo[:])
    nc.finalize()
    return nc


def _fold(w, g, b):
    s = (np.asarray(g, np.float32) / np.sqrt(np.float32(1.0 + _EPS)))
    return np.asarray(w, np.float32) * s[:, None], np.asarray(b, np.float32)


def kernel(feature, xyz, w_mlp1, g_mlp1, b_mlp1, bb_w1, bb_g1, bb_b1,
           att1_fc, att1_w, att1_g, att1_b, bb_w2, bb_g2, bb_b2,
           att2_fc, att2_w, att2_g, att2_b, w_mlp2, g_mlp2, b_mlp2,
           w_sc, g_sc, b_sc, neigh_idx):
    from concourse.bass_utils import run_bass_kernel_spmd

    feature = np.asarray(feature, np.float32)
    xyz = np.asarray(xyz, np.float32)
    idx = np.asarray(neigh_idx).astype(np.int64)
    trace = bool(int(os.environ.get("RANDLA_TRACE", "0")))

    W1f, B1 = _fold(w_mlp1, g_mlp1, b_mlp1)
    Wb1f, Bb1 = _fold(bb_w1, bb_g1, bb_b1)
    Wa1f, Ba1 = _fold(att1_w, att1_g, att1_b)
    Wb2f, Bb2 = _fold(bb_w2, bb_g2, bb_b2)
    Wa2f, Ba2 = _fold(att2_w, att2_g, att2_b)
    Wm2f, Bm2 = _fold(w_mlp2, g_mlp2, b_mlp2)
    Wscf, Bsc = _fold(w_sc, g_sc, b_sc)
    Bout = Bm2 + Bsc
    A = Wb1f[:, 1:4] + Wb1f[:, 4:7]
    C = -Wb1f[:, 1:4] + Wb1f[:, 7:10]
    wd = Wb1f[:, 0]
    fc1 = np.asarray(att1_fc, np.float32)
    fc2 = np.asarray(att2_fc, np.float32)

    w1aug = np.concatenate([W1f.T, B1[None, :]], 0).astype(BF16)
    # combo8 rows must match rhs rows [nx(3); ones; dist; xtile(3)]
    combo8 = np.concatenate(
        [C.T, Bb1[None, :], wd[None, :], A.T], 0).astype(BF16)
    wmscT = np.concatenate([Wm2f.T, Wscf.T, Bout[None, :]], 0).astype(BF16)

    ins1, idxws, feats = [], [], []
    for c in range(8):
        b, q = c // 4, c % 4
        featb = np.ascontiguousarray(feature[b, :, :, 0])            # [32, N]
        feat33 = np.concatenate([featb, np.ones((1, N), np.float32)], 0)
        xyzb = np.ascontiguousarray(xyz[b]).astype(BF16)             # [N, 3]
        flat = idx[b, q * NL:(q + 1) * NL].reshape(-1).astype(np.int16)
        idxw = np.ascontiguousarray(
            np.tile(flat.reshape(E // 16, 16).T, (8, 1)))
        feats.append(feat33)
        idxws.append(idxw)
        xyzw = np.ascontiguousarray(
            xyzb.reshape(N // 128, 128, 3).transpose(1, 0, 2))
        ins1.append({
            "feat33": feat33.astype(BF16), "xyzw": xyzw,
            "xyzq": np.ascontiguousarray(xyzb[q * NL:(q + 1) * NL]), "idxw": idxw,
            "w1aug": w1aug, "combo8": combo8,
            "fc1T": np.ascontiguousarray(fc1.T).astype(BF16),
            "wa1T": np.ascontiguousarray(Wa1f.T).astype(BF16),
            "ba1": np.ascontiguousarray(Ba1[:, None]),
        })

    nc1 = build_launch1()
    r1 = run_bass_kernel_spmd(nc1, ins1, core_ids=list(range(8)), trace=trace)
    last_exec_ns[0] = r1.exec_time_ns or 0

    fagg_full = [
        np.concatenate([r1.results[b * 4 + q]["f_agg"] for q in range(4)], 1)
        for b in range(2)
    ]  # [32, N] fp32 per batch

    ins2 = []
    for c in range(8):
        b, q = c // 4, c % 4
        ins2.append({
            "faggT": fagg_full[b].astype(BF16), "fxyz": r1.results[c]["fxyz"],
            "feat33l": np.ascontiguousarray(
                feats[c][:, q * NL:(q + 1) * NL]).astype(BF16),
            "idxw": idxws[c],
            "wb2T": np.ascontiguousarray(Wb2f.T).astype(BF16),
            "bb2": np.ascontiguousarray(Bb2[:, None]),
            "fc2T": np.ascontiguousarray(fc2.T).astype(BF16),
            "wa2T": np.ascontiguousarray(Wa2f.T).astype(BF16),
            "ba2": np.ascontiguousarray(Ba2[:, None]),
            "wmscT": wmscT,
        })

    nc2 = build_launch2()
    r2 = run_bass_kernel_spmd(nc2, ins2, core_ids=list(range(8)), trace=trace)
    last_exec_ns[1] = r2.exec_time_ns or 0

    out = np.empty((2, 128, N, 1), np.float32)
    for c in range(8):
        b, q = c // 4, c % 4
        out[b, :, q * NL:(q + 1) * NL, 0] = r2.results[c]["out"]
    return out
